# revision 1
# baseline (speedup 1.0000x reference)
import sys

import numpy as np

for _p in ("/opt/trn_rl_repo",):
    if _p not in sys.path:
        sys.path.insert(0, _p)

import ml_dtypes

import concourse.bass as bass
import concourse.bacc as bacc
import concourse.mybir as mybir
import concourse.tile as tile
from concourse.bass_utils import run_bass_kernel_spmd
from bass_rust import AP

F32 = mybir.dt.float32
BF16 = mybir.dt.bfloat16
BF = ml_dtypes.bfloat16

# Problem dims (hardcoded per contract)
B, S, E, H = 2, 2048, 512, 32
D = E // H            # 16
NCORE = 8
HPC = H // NCORE      # 4 heads per core
C = HPC * D           # 64 channels per core
JCOLS = 2432          # window row length: offsets j0 in {0,128,...,1920} + 512
PLEN = 2559           # padded toeplitz row: 511 zeros + 2048 softplus'd weights
NT = S // 512         # 4 t-blocks of 512

_CACHE = {}

# 16 x-chunks of 1024 cols (2 s-tiles each); alternate batches so both
# batches' s-tiles arrive early, unblocking mixing block tj after 4(tj+1)
CHUNK_ORDER = tuple(
    b * 8 + j for j in range(8) for b in range(2)
)


def _build_program():
    nc = bacc.Bacc()

    xT_d = nc.dram_tensor("xT", [128, 8 * 2048], BF16, kind="ExternalInput")
    inpwT_d = nc.dram_tensor("inpwT", [E, C], BF16, kind="ExternalInput")
    biasT_d = nc.dram_tensor("biasT", [128, C], BF16, kind="ExternalInput")
    wpad_d = nc.dram_tensor("wpad", [HPC, PLEN], BF16, kind="ExternalInput")
    rnrx_d = nc.dram_tensor("rnrx", [128, S], BF16, kind="ExternalInput")
    womask_d = nc.dram_tensor("womask", [B, 128, E], BF16, kind="ExternalInput")
    out_d = nc.dram_tensor("out", [B, NT, 128, 4 * 512], BF16, kind="ExternalOutput")

    with tile.TileContext(nc) as tc:
        with (
            tc.tile_pool(name="persist", bufs=1) as pp,
            tc.tile_pool(name="wstage", bufs=3) as wstage,
            tc.tile_pool(name="psh", bufs=2, space="PSUM") as psh,
            tc.tile_pool(name="psy", bufs=2, space="PSUM") as psy,
            tc.tile_pool(name="pso", bufs=4, space="PSUM") as pso,
        ):
            # ---- PE warmup: fill the DMA startup bubble with dummy
            # matmuls on a zeroed tile so the ramp (HAM) is warm when the
            # first real matmul issues ----
            wz = pp.tile([128, 512], BF16, tag="wz")
            nc.gpsimd.memset(wz[:], 0.0)

            def warm(n):
                for _ in range(n):
                    pw = pso.tile([128, 512], F32, tag="po", name="pw")
                    nc.tensor.matmul(
                        pw[:], wz[:, 0:128], wz[:], start=True, stop=True
                    )

            warm(6)

            # ---- x chunks first on the SP (sync) HWDGE ring ----
            xk = pp.tile([128, 8 * 2048], BF16, tag="xk", name="xk")
            for i, q in enumerate(CHUNK_ORDER):
                if i < 8:
                    nc.sync.dma_start(
                        xk[:, q * 1024 : (q + 1) * 1024],
                        xT_d[:, q * 1024 : (q + 1) * 1024],
                    )
                elif q % 2 == 0:  # late chunks: coarse 2048-col DMAs
                    nc.sync.dma_start(
                        xk[:, q * 1024 : (q + 2) * 1024],
                        xT_d[:, q * 1024 : (q + 2) * 1024],
                    )

            # ---- weights/constants on the ACT (scalar) HWDGE ring ----
            wt = pp.tile([128, 4, C], BF16, tag="wt")  # inp_wT as (p, k, c)
            nc.scalar.dma_start(
                wt[:], inpwT_d[:].rearrange("(k p) c -> p k c", p=128)
            )
            biasT = pp.tile([128, C], BF16, tag="biasT")
            nc.scalar.dma_start(biasT[:], biasT_d[:])

            # ---- shifted Toeplitz rows via overlapping-window DMA ----
            # mstar[g][p, j] = wpad[g, p + j + 384]; wpad = [511 zeros,
            # softplus(w_g)]. Cols j < 384 of the full window are never read
            # (provably zero), so tiles cover j in [384, 2432) only.
            # Two column slices per head: tj<=1 needs j < 1408.
            mstar = []
            for g in range(HPC):
                mt = pp.tile([128, 2048], BF16, tag=f"mstar{g}", name=f"mstar{g}")
                nc.scalar.dma_start(
                    mt[:, 0:1024],
                    AP(wpad_d, g * PLEN + 384, [[1, 128], [1, 1024]]),
                )
                mstar.append(mt)
            wom = []
            for b in range(B):
                t = pp.tile([128, E], BF16, tag=f"wom{b}")
                nc.scalar.dma_start(t[:], womask_d[b])
                wom.append(t)
            rnrx = pp.tile([128, S], BF16, tag="rnrx")
            nc.scalar.dma_start(rnrx[:], rnrx_d[:])

            def load_mstar_hi():
                for g in range(HPC):
                    nc.scalar.dma_start(
                        mstar[g][:, 1024:2048],
                        AP(wpad_d, g * PLEN + 1408, [[1, 128], [1, 1024]]),
                    )

            # hT[si]: [s 128 (reversed within tile), (g, b, d) 128] bf16
            h_sb = [
                pp.tile([128, 128], BF16, tag=f"h{i}", name=f"h{i}")
                for i in range(S // 128)
            ]
            y_sb = [
                pp.tile([128, 512], BF16, tag=f"y{j}", name=f"y{j}")
                for j in range(NT)
            ]

            def proj_chunk(q):
                # chunk q covers batch b = q//8, s-tiles 2*(q%8), 2*(q%8)+1
                b = q // 8
                for u2 in range(2):
                    si = (q % 8) * 2 + u2
                    ph = psh.tile([128, C], F32, name="ph")
                    for k in range(4):
                        c0 = q * 1024 + k * 256 + u2 * 128
                        nc.tensor.matmul(
                            ph[:],
                            xk[:, c0 : c0 + 128],
                            wt[:, k, :],
                            start=(k == 0),
                            stop=(k == 3),
                        )
                    dst = h_sb[si][:].rearrange(
                        "p (g b2 d) -> p g b2 d", g=HPC, b2=2
                    )[:, :, b, :]
                    nc.vector.tensor_add(
                        dst,
                        ph[:].rearrange("p (g d) -> p g d", g=HPC),
                        biasT[:].rearrange("p (g d) -> p g d", g=HPC),
                    )

            def mix_half(tj, h, py):
                # half h covers output cols [256h, 256h+256)
                nsi = 4 * tj + 4
                lo, hi = 256 * h, 256 * h + 256
                for si in range(nsi):
                    j0 = 512 * tj - 128 * si
                    # si > 4tj windows: output cols n < -j0 are provably zero
                    n0 = max(lo, -j0)
                    if n0 >= hi:
                        continue
                    for g in range(HPC):
                        nc.tensor.matmul(
                            py[32 * g : 32 * g + 32, n0:hi],
                            h_sb[si][:, 32 * g : 32 * g + 32],
                            mstar[g][:, j0 + n0 : j0 + hi],
                            start=(si == 0),
                            stop=(si == nsi - 1),
                            tile_position=(0, 32 * g),
                            skip_group_check=True,
                        )
                nc.vector.tensor_mul(
                    y_sb[tj][:, lo:hi],
                    py[:, lo:hi],
                    rnrx[:, 512 * tj + lo : 512 * tj + hi],
                )

            def outproj_half(tj, h, osts):
                # consumes y_sb[tj] t-tiles 2h, 2h+1; writes half-ost + DMA
                for b in range(B):
                    for tt in (2 * h, 2 * h + 1):
                        po = pso.tile([128, 512], F32, name="po")
                        nc.tensor.matmul(
                            po[:],
                            y_sb[tj][:, tt * 128 : tt * 128 + 128],
                            wom[b][:],
                            start=True,
                            stop=True,
                        )
                        if (tt + b) % 2 == 0:
                            nc.vector.tensor_copy(
                                osts[b][:, tt * 512 : (tt + 1) * 512], po[:]
                            )
                        else:
                            nc.scalar.copy(
                                osts[b][:, tt * 512 : (tt + 1) * 512], po[:]
                            )
                    nc.sync.dma_start(
                        out_d[b, tj, :, 1024 * h : 1024 * h + 1024],
                        osts[b][:, 1024 * h : 1024 * h + 1024],
                    )

            # interleave: proj chunks feed mixing halves; outproj of half 0
            # overlaps mixing of half 1
            pending = None  # (tj, osts) whose outproj is deferred
            for tj in range(NT):
                if tj == 1:
                    load_mstar_hi()
                for q in CHUNK_ORDER[4 * tj : 4 * tj + 4]:
                    proj_chunk(q)
                if pending is not None:
                    ptj, posts = pending
                    outproj_half(ptj, 0, posts)
                    outproj_half(ptj, 1, posts)
                py = psy.tile([128, 512], F32, name="py")
                osts = [
                    wstage.tile([128, 4 * 512], BF16, tag=f"ost{b}", name=f"ost{b}")
                    for b in range(B)
                ]
                mix_half(tj, 0, py)
                mix_half(tj, 1, py)
                pending = (tj, osts)
            ptj, posts = pending
            outproj_half(ptj, 0, posts)
            outproj_half(ptj, 1, posts)
    nc.compile()
    return nc


def _softplus(v):
    return np.log1p(np.exp(-np.abs(v))) + np.maximum(v, 0.0)


def _host_prep(x, weight_raw, bias, inp_w, inp_b, out_w):
    x = np.asarray(x, np.float32)
    weight_raw = np.asarray(weight_raw, np.float32)
    bias = np.asarray(bias, np.float32)
    inp_w = np.asarray(inp_w, np.float32)
    inp_b = np.asarray(inp_b, np.float32)
    out_w = np.asarray(out_w, np.float32)

    spw = _softplus(weight_raw[:, :S])          # (H, S)
    rinv = 1.0 / np.cumsum(spw, axis=1)         # (H, S)

    # x -> [p, (b, sb4, half, k, u2, r)] with s reversed within each 128-tile;
    # each 1024-col chunk (b, sb4, half) holds 2 s-tiles x 4 e-slabs
    sidx = np.arange(S).reshape(S // 128, 128)[:, ::-1].reshape(S)
    xr = x[:, sidx, :]                          # (B, S, E), s' order
    # dims: [b, sb4, half, u2, r, k, p] -> [p, b, sb4, half, k, u2, r]
    xTd = np.ascontiguousarray(
        xr.reshape(B, 4, 2, 2, 128, 4, 128)
        .transpose(6, 0, 1, 2, 5, 3, 4)
        .reshape(128, 8 * 2048)
    ).astype(BF)

    in_maps = []
    for core in range(NCORE):
        c0 = core * C
        heads = slice(core * HPC, (core + 1) * HPC)

        wpad = np.zeros((HPC, PLEN), np.float32)
        wpad[:, 511:] = spw[heads]

        # rnrx[(g, b, d), t] = rinv[head g, t]
        rnrx = np.repeat(rinv[heads], 32, axis=0)  # (128, S)

        womask = np.zeros((B, 128, E), np.float32)
        wo_slice = out_w[:, c0 : c0 + C].T         # (C=(g,d), E)
        for b in range(B):
            v = womask[b].reshape(HPC, 2, D, E)
            v[:, b, :, :] = wo_slice.reshape(HPC, D, E)

        in_maps.append(
            {
                "xT": xTd,
                "inpwT": np.ascontiguousarray(inp_w[c0 : c0 + C, :].T).astype(BF),
                "biasT": np.broadcast_to(
                    inp_b[c0 : c0 + C], (128, C)
                ).astype(BF).copy(),
                "wpad": wpad.astype(BF),
                "rnrx": rnrx.astype(BF),
                "womask": womask.astype(BF),
            }
        )

    # input-independent bias contribution, added on host:
    # biasout[t, e] = sum_c bias[head(c), t] * out_w[e, c]
    w2 = out_w.reshape(E, H, D).sum(2)             # (E, H)
    biasout = bias[:, :S].T @ w2.T                  # (S, E)
    return in_maps, biasout


def _run(in_maps, trace=False):
    if "nc" not in _CACHE:
        _CACHE["nc"] = _build_program()
    try:
        res = run_bass_kernel_spmd(
            _CACHE["nc"], in_maps, core_ids=list(range(NCORE)), trace=trace
        )
    except ModuleNotFoundError:
        res = run_bass_kernel_spmd(
            _CACHE["nc"], in_maps, core_ids=list(range(NCORE)), trace=False
        )
    return res


def kernel(x, weight_raw, bias, inp_w, inp_b, out_w, parallel=True, _trace=False):
    in_maps, biasout = _host_prep(x, weight_raw, bias, inp_w, inp_b, out_w)
    res = _run(in_maps, trace=_trace)
    out = np.zeros((B, S, E), np.float32)
    for r in res.results:
        o = np.asarray(r["out"], dtype=np.float32)  # (B, NT, 128, 2048)
        out += o.reshape(B, 4, 128, 4, 512).transpose(0, 1, 3, 2, 4).reshape(B, S, E)
    out += biasout[None]
    if _trace:
        kernel.last_exec_ns = res.exec_time_ns
        kernel.last_results = res
    return out


if __name__ == "__main__":
    rng = np.random.default_rng(0)
    inputs = {
        "x": rng.standard_normal((B, S, E)).astype(np.float32),
        "weight_raw": rng.standard_normal((H, S)).astype(np.float32),
        "bias": np.zeros((H, S), np.float32),
        "inp_w": (rng.standard_normal((E, E)) / np.sqrt(E)).astype(np.float32),
        "inp_b": np.zeros((E,), np.float32),
        "out_w": (rng.standard_normal((E, E)) / np.sqrt(E)).astype(np.float32),
    }
    o = kernel(**inputs)
    print("ok", o.shape, float(np.abs(o).mean()))

